# revision 1
# baseline (speedup 1.0000x reference)
"""Trainium2 Bass kernel for nn_DIDAModuleD4 (dynamic depthwise conv module).

Data-parallel over batch: 32 samples -> 8 cores x 4 samples.
Per core, samples are processed in 2 blocks of 2 samples; each block maps the
2x64=128 (sample, channel) pairs onto the 128 SBUF partitions.

Math (per sample, with host-side weight folding):
  f   = relu(conv_w @ x + conv_b)                       [64, 4096]
  g   = relu(mean_px(conv_w @ x + conv_b))              [64]
  k_t = a_t * g + b_t            (43 taps, a/b host-folded scalars)
  o_i = sum_t k_t * shift_t(f)   (depthwise; 5x5, 3x3 d2, 3x3 d4)
  out = sum_i W_i @ o_i + bias_out                      [384, 4096]
        where W_i = fc_w[:, 128i:128(i+1)] @ fuse_w  (host-folded)

Engine split (cost-model driven: PE matmul = N*0.42ns regardless of K/M,
DVE ts 4x + tt 2x = 0.78ns/elem, Pool ~2x slower than DVE -> no Pool taps):
  - conv1x1: float32r matmuls; one [128, 512] PSUM tile per chunk holds both
    samples (M=64 writes to partition halves); ACT evacuates with bias and
    accum_out for the g means.
  - 5x5 branch (25 taps): diagonal-matmul PSUM accumulation on PE, taps
    iterated OUTER over 2-chunk groups so each diag Ldweights is shared by
    2 matmuls (PE sequencer relief).
  - 3x3 dil2/dil4 branches (18 taps): DVE tensor_scalar (4x) +
    tensor_tensor (2x) on 2048-px half-images; the last accumulate is split
    into two 64-partition adds that write per-sample branch-pair tiles
    X=(o2_s0;o3_s0), Y=(o2_s1;o3_s1) -> K=128 out matmuls.
  - output: per (sample, mt) psum accumulates one K=128 matmul (X/Y against
    branch-packed wout12) and one K=64 matmul (o1 slice against wout1);
    ACT adds bias on evacuation; stores issued from the ACT sequencer.
Datapath dtypes: conv f32r; f_pad/diag/o/wout bf16; psum fp32; I/O fp32.
"""

import sys

if "/opt/trn_rl_repo" not in sys.path:
    sys.path.insert(0, "/opt/trn_rl_repo")

import os
import numpy as np
from contextlib import ExitStack

from concourse import bass, mybir, tile, bacc
from concourse.bass_utils import run_bass_kernel_spmd

F32 = mybir.dt.float32
F32R = mybir.dt.float32r
BF16 = mybir.dt.bfloat16
AF = mybir.ActivationFunctionType
ALU = mybir.AluOpType

N_CORES = 8
SAMPLES_PER_CORE = 4
CM = 64          # reduced channels / groups
CIN = 256
COUT = 384
H = W = 64
PIX = H * W      # 4096
PAD = 4
WP = W + 2 * PAD  # 72
HALF = 2048      # pixels per half-image (32 rows)
NHALF = PIX // HALF          # 2
CHUNK = 512                  # matmul N (one PSUM bank)
NCHUNK = PIX // CHUNK        # 8
CPH = HALF // CHUNK          # chunks per half (4)
GRP = 2                      # chunks per tap-outer group
SLAB = 1024
NSLAB = PIX // SLAB

# taps: (branch, dy, dx, dilation); ktile column order must match aT/bT
TAPS = (
    [(0, dy, dx, 1) for dy in range(-2, 3) for dx in range(-2, 3)]
    + [(1, dy, dx, 2) for dy in range(-1, 2) for dx in range(-1, 2)]
    + [(2, dy, dx, 4) for dy in range(-1, 2) for dx in range(-1, 2)]
)
NTAP = len(TAPS)  # 43

TENSOR_TAPS = [t for t in range(NTAP) if TAPS[t][0] == 0]   # 25, on PE
DVE_B1 = [t for t in range(NTAP) if TAPS[t][0] == 1]        # 9, on DVE
DVE_B2 = [t for t in range(NTAP) if TAPS[t][0] == 2]        # 9, on DVE

_PROGRAM_CACHE = {}


def _fpad_view(fp_t, r0, nrows, off_r, off_c, dtype=None):
    """View of padded-f tile [128, WP*WP] covering output rows [r0, r0+nrows)
    shifted by (off_r, off_c). Returns [128, nrows, 64] AP."""
    v = fp_t[:].rearrange("p (r c) -> p r c", c=WP)
    if dtype is not None:
        v = v.bitcast(dtype)
    return v[:, PAD + r0 + off_r : PAD + r0 + nrows + off_r,
             PAD + off_c : PAD + W + off_c]


def _build_program():
    nc = bacc.Bacc("TRN2", target_bir_lowering=False, debug=False,
                   num_devices=N_CORES)

    x4 = nc.dram_tensor("x4", [SAMPLES_PER_CORE, CIN, PIX], BF16,
                        kind="ExternalInput").ap()
    wconv = nc.dram_tensor("wconv", [128, 512], BF16,
                           kind="ExternalInput").ap()
    wout12_d = nc.dram_tensor("wout12", [128, 3 * 128], BF16,
                              kind="ExternalInput").ap()
    wout1_d = nc.dram_tensor("wout1", [128, COUT], BF16,
                             kind="ExternalInput").ap()
    aT_d = nc.dram_tensor("aT", [128, NTAP], F32, kind="ExternalInput").ap()
    bT_d = nc.dram_tensor("bT", [128, NTAP], F32, kind="ExternalInput").ap()
    ident_d = nc.dram_tensor("ident", [128, 128], F32,
                             kind="ExternalInput").ap()
    convb_d = nc.dram_tensor("convb", [128, 1], F32, kind="ExternalInput").ap()
    biasout_d = nc.dram_tensor("biasout", [128, 3], F32,
                               kind="ExternalInput").ap()
    y4 = nc.dram_tensor("y4", [SAMPLES_PER_CORE, COUT, PIX], F32,
                        kind="ExternalOutput").ap()

    with tile.TileContext(nc) as tc:
        with ExitStack() as ctx:
            consts = ctx.enter_context(tc.tile_pool(name="consts", bufs=1))
            xpool = ctx.enter_context(tc.tile_pool(name="xp", bufs=3))
            fpool = ctx.enter_context(tc.tile_pool(name="fp", bufs=1))
            opool = ctx.enter_context(tc.tile_pool(name="op", bufs=2))
            o1pool = ctx.enter_context(tc.tile_pool(name="o1p", bufs=2))
            outpool = ctx.enter_context(tc.tile_pool(name="outp", bufs=2))
            smalls = ctx.enter_context(tc.tile_pool(name="sm", bufs=2))
            diagp = ctx.enter_context(tc.tile_pool(name="dg", bufs=2))
            # PSUM: pool A = 2 banks (conv pairs + o1 tap groups),
            #       pool B = 6 banks (out psums per (s, mt)).
            ps_a = ctx.enter_context(
                tc.tile_pool(name="psa", bufs=1, space="PSUM"))
            ps_out = ctx.enter_context(
                tc.tile_pool(name="pso", bufs=1, space="PSUM"))

            # ---- constants (issued on the GPSIMD sequencer) ----
            wconv_t = consts.tile([128, 512], BF16, tag="wconv")
            nc.gpsimd.dma_start(wconv_t[:], wconv[:])
            wout12_t = consts.tile([128, 3 * 128], BF16, tag="wout12")
            nc.gpsimd.dma_start(wout12_t[:], wout12_d[:])
            wout1_t = consts.tile([128, COUT], BF16, tag="wout1")
            nc.gpsimd.dma_start(wout1_t[:], wout1_d[:])
            aT = consts.tile([128, NTAP], F32, tag="aT")
            nc.gpsimd.dma_start(aT[:], aT_d[:])
            bT = consts.tile([128, NTAP], F32, tag="bT")
            nc.gpsimd.dma_start(bT[:], bT_d[:])
            ident = consts.tile([128, 128], F32, tag="ident")
            nc.gpsimd.dma_start(ident[:], ident_d[:])
            convb = consts.tile([128, 1], F32, tag="convb")
            nc.gpsimd.dma_start(convb[:], convb_d[:])
            biasout = consts.tile([128, 3], F32, tag="biasout")
            nc.gpsimd.dma_start(biasout[:], biasout_d[:])

            # persistent padded-f tiles (one per block parity); borders are
            # zeroed once and never rewritten (interior writes only).
            zeros = consts.tile([128, PAD * WP], F32, tag="zeros")
            nc.gpsimd.memset(zeros[:], 0.0)
            fpads = []
            for par in range(2):
                fp_t = fpool.tile([128, WP * WP], BF16, tag=f"fpad{par}")
                v = fp_t[:].rearrange("p (r c) -> p r c", c=WP)
                nc.vector.tensor_copy(fp_t[:, 0:PAD * WP], zeros[:])
                nc.vector.tensor_copy(fp_t[:, (PAD + H) * WP:WP * WP],
                                      zeros[:])
                zv = zeros[:].rearrange("p (r c) -> p r c", c=PAD)
                nc.vector.tensor_copy(v[:, PAD:PAD + H, 0:PAD],
                                      zv[:, 0:H, :])
                nc.vector.tensor_copy(v[:, PAD:PAD + H, PAD + W:WP],
                                      zv[:, 0:H, :])
                fpads.append(fp_t)

            # ---- per-block emitters ----
            def emit_conv(blk):
                """conv + g + ktile + diag tiles for one sample pair."""
                n0, n1 = 2 * blk, 2 * blk + 1
                fp_t = fpads[blk % 2]

                gsums = smalls.tile([128, NCHUNK], F32, tag=f"gsums{blk}")
                for q in range(NSLAB):
                    # x slab tiles: partitions = (sample pair, 64-chan
                    # group); block-diag wconv contracts both samples in
                    # one K=128 matmul per 64-channel group.
                    xts = {}
                    for kc in range(4):
                        xt = xpool.tile([128, SLAB], BF16, tag=f"x{kc}")
                        nc.sync.dma_start(
                            xt[:],
                            x4[n0:n0 + 2, kc * 64:(kc + 1) * 64,
                               q * SLAB:(q + 1) * SLAB])
                        xts[kc] = xt
                    for c in range(SLAB // CHUNK):
                        j = q * (SLAB // CHUNK) + c  # global chunk index
                        ps = ps_a.tile([128, CHUNK], F32,
                                       tag=f"psa{j % GRP}")
                        for kc in range(4):
                            nc.tensor.matmul(
                                ps[:],
                                wconv_t[:, kc * 128:(kc + 1) * 128],
                                xts[kc][:, c * CHUNK:(c + 1) * CHUNK],
                                start=(kc == 0), stop=(kc == 3))
                        # evacuate with bias; accum_out gives sum for g
                        dst = _fpad_view(fp_t, 8 * j, 8, 0, 0)
                        nc.scalar.activation(
                            dst, ps[:], AF.Identity,
                            bias=convb[:, 0:1],
                            accum_out=gsums[:, j:j + 1])

                # in-place relu over the interior
                intr = _fpad_view(fp_t, 0, H, 0, 0)
                nc.vector.tensor_scalar_max(intr, intr, 0.0)

                # ---- g, ktile, diag tiles ----
                gpre = smalls.tile([128, 1], F32, tag=f"gpre{blk}")
                nc.vector.tensor_reduce(gpre[:], gsums[:], op=ALU.add,
                                        axis=mybir.AxisListType.X)
                gt = smalls.tile([128, 1], F32, tag=f"g{blk}")
                nc.scalar.activation(gt[:], gpre[:], AF.Relu,
                                     scale=1.0 / PIX)
                ktile = smalls.tile([128, NTAP], F32, tag=f"ktile{blk}")
                nc.vector.scalar_tensor_tensor(ktile[:], aT[:], gt[:, 0:1],
                                               bT[:], op0=ALU.mult,
                                               op1=ALU.add)

                diags = {}
                for t in TENSOR_TAPS:
                    dg = diagp.tile([128, 128], BF16, tag=f"diag{t}")
                    nc.gpsimd.tensor_scalar_mul(dg[:], ident[:],
                                                ktile[:, t:t + 1])
                    diags[t] = dg
                return (fp_t, ktile, diags, n0, n1)

            def emit_out(st):
                """Output matmuls + evac + store for one (blk, h)."""
                fp_t, ktile, n0, n1, h, o1_t, Xt, Yt = st
                osbs = {}
                for g2 in range(CPH // GRP):       # 1024-px store groups
                    for mt in range(3):
                        for s in range(2):
                            osb_tile = outpool.tile(
                                [128, GRP * CHUNK], F32, tag=f"osb{mt}_{s}")
                            osbs[(mt, s)] = osb_tile
                    for cc in range(GRP):
                        c = g2 * GRP + cc          # chunk within half
                        csl = slice(c * CHUNK, (c + 1) * CHUNK)
                        pss = {}
                        # K=128 branch-pair matmuls (lhsT shared across s)
                        for mt in range(3):
                            for s, bt in ((0, Xt), (1, Yt)):
                                ps = ps_out.tile([128, CHUNK], F32,
                                                 tag=f"out{s}_{mt}")
                                pss[(s, mt)] = ps
                                nc.tensor.matmul(
                                    ps[:],
                                    wout12_t[:, mt * 128:(mt + 1) * 128],
                                    bt[:, csl], start=True, stop=False)
                        # K=64 o1 matmuls close the accumulation
                        for mt in range(3):
                            for s in range(2):
                                sl = slice(64 * s, 64 * s + 64)
                                nc.tensor.matmul(
                                    pss[(s, mt)][:],
                                    wout1_t[sl, mt * 128:(mt + 1) * 128],
                                    o1_t[sl, csl], start=False, stop=True)
                        for mt in range(3):
                            for s in range(2):
                                nc.scalar.activation(
                                    osbs[(mt, s)][:, cc * CHUNK:
                                                  (cc + 1) * CHUNK],
                                    pss[(s, mt)][:], AF.Identity,
                                    bias=biasout[:, mt:mt + 1])
                    px0 = h * HALF + g2 * GRP * CHUNK
                    for mt in range(3):
                        for s in range(2):
                            n = (n0, n1)[s]
                            dst = y4[n, mt * 128:(mt + 1) * 128,
                                     px0:px0 + GRP * CHUNK]
                            # split stores between the HWDGE queue (ACT
                            # issue) and the SWDGE path (Pool engine) —
                            # the two run in parallel in the DMA model
                            if g2 == 0:
                                nc.gpsimd.dma_start(dst, osbs[(mt, s)][:])
                            else:
                                nc.scalar.dma_start(dst, osbs[(mt, s)][:])

            def emit_taps(blk_st, h):
                fp_t, ktile, diags, n0, n1 = blk_st
                if True:
                    r0 = h * (HALF // W)       # first output row (32/half)
                    nr = HALF // W             # rows per half (32)

                    # ---- branch 0 (5x5) on PE: tap-outer over 2-chunk
                    # groups so each diag Ldweights serves GRP matmuls ----
                    o1_t = o1pool.tile([128, HALF], BF16, tag="o1")
                    for g2 in range(CPH // GRP):
                        pso = []
                        for i in range(GRP):
                            pso_i = ps_a.tile([128, CHUNK], F32,
                                              tag=f"psa{i}")
                            pso.append(pso_i)
                        for i, t in enumerate(TENSOR_TAPS):
                            _, dy, dx, dil = TAPS[t]
                            last = (i == len(TENSOR_TAPS) - 1)
                            for cc in range(GRP):
                                c = g2 * GRP + cc
                                rhs = _fpad_view(
                                    fp_t, r0 + c * (CHUNK // W),
                                    CHUNK // W, dy * dil, dx * dil)
                                nc.tensor.matmul(pso[cc][:], diags[t][:],
                                                 rhs, start=(i == 0),
                                                 stop=last)
                        for cc in range(GRP):
                            c = g2 * GRP + cc
                            nc.scalar.activation(
                                o1_t[:, c * CHUNK:(c + 1) * CHUNK],
                                pso[cc][:], AF.Copy)

                    # ---- branches 1,2 on DVE -> X=(o2s0;o3s0) Y=(o2s1;o3s1)
                    Xt = opool.tile([128, HALF], BF16, tag="X")
                    Yt = opool.tile([128, HALF], BF16, tag="Y")
                    for bi, btaps in ((0, DVE_B1), (1, DVE_B2)):
                        psl = slice(64 * bi, 64 * bi + 64)
                        acc = opool.tile([128, HALF], BF16, tag=f"acc{bi}")
                        av = acc[:].rearrange("p (r c) -> p r c", c=W)
                        for i, t in enumerate(btaps):
                            _, dy, dx, dil = TAPS[t]
                            src = _fpad_view(fp_t, r0, nr, dy * dil,
                                             dx * dil)
                            if i == 0:
                                nc.vector.tensor_scalar_mul(
                                    av, src, ktile[:, t:t + 1])
                                continue
                            tmp = opool.tile([128, HALF], BF16, tag="tmp")
                            tv = tmp[:].rearrange("p (r c) -> p r c", c=W)
                            nc.vector.tensor_scalar_mul(
                                tv, src, ktile[:, t:t + 1])
                            if i < len(btaps) - 1:
                                nc.vector.tensor_tensor(
                                    out=av, in0=av, in1=tv, op=ALU.add)
                            else:
                                # split final accumulate into per-sample
                                # halves written to the branch-pair tiles
                                nc.vector.tensor_tensor(
                                    out=Xt[psl, :], in0=acc[0:64, :],
                                    in1=tmp[0:64, :], op=ALU.add)
                                nc.vector.tensor_tensor(
                                    out=Yt[psl, :], in0=acc[64:128, :],
                                    in1=tmp[64:128, :], op=ALU.add)

                    return (fp_t, ktile, n0, n1, h, o1_t, Xt, Yt)

            # ---- schedule: conv(b1) is deferred until after the first
            # tap block so its x loads aren't starved by b0's, and each
            # out(blk, h) is emitted after the taps of the NEXT (blk, h)
            # so PE never waits on the DVE branch tiles ----
            st0 = emit_conv(0)
            p00 = emit_taps(st0, 0)
            st1 = emit_conv(1)
            p01 = emit_taps(st0, 1)
            emit_out(p00)
            p10 = emit_taps(st1, 0)
            emit_out(p01)
            p11 = emit_taps(st1, 1)
            emit_out(p10)
            emit_out(p11)
    nc.compile()
    return nc


def _get_program():
    if "nc" not in _PROGRAM_CACHE:
        _PROGRAM_CACHE["nc"] = _build_program()
    return _PROGRAM_CACHE["nc"]


def kernel(x, conv_w, conv_b, ck_w, ck_b, ck2_w, ck2_b, ckd4_w, ckd4_b,
           kern_w, kern_b, kern2_w, kern2_b, kernd4_w, kernd4_b,
           fuse_w, fuse_b, fc_w, fc_b):
    x = np.asarray(x, dtype=np.float32)
    conv_w = np.asarray(conv_w, dtype=np.float32)
    conv_b = np.asarray(conv_b, dtype=np.float32)
    fuse_w = np.asarray(fuse_w, dtype=np.float32)
    fuse_b = np.asarray(fuse_b, dtype=np.float32)
    fc_w = np.asarray(fc_w, dtype=np.float32)
    fc_b = np.asarray(fc_b, dtype=np.float32)

    NB = x.shape[0]
    assert NB == N_CORES * SAMPLES_PER_CORE

    # ---- host-side weight folding ----
    # tap affine coefficients: k_t = a_t * g + b_t
    a1 = (float(ck_w) * np.asarray(kern_w)).astype(np.float32)        # [25]
    b1 = (float(ck_w) * np.asarray(kern_b) + float(ck_b)).astype(np.float32)
    a2 = (float(ck2_w) * np.asarray(kern2_w)).astype(np.float32)      # [9]
    b2 = (float(ck2_w) * np.asarray(kern2_b) + float(ck2_b)).astype(np.float32)
    a3 = (float(ckd4_w) * np.asarray(kernd4_w)).astype(np.float32)    # [9]
    b3 = (float(ckd4_w) * np.asarray(kernd4_b) + float(ckd4_b)).astype(np.float32)
    a_all = np.concatenate([a1, a2, a3]).astype(np.float32)           # [43]
    b_all = np.concatenate([b1, b2, b3]).astype(np.float32)
    aT = np.broadcast_to(a_all, (128, NTAP)).copy()
    bT = np.broadcast_to(b_all, (128, NTAP)).copy()

    # folded output weights W_i = fc_w[:, 128i:128(i+1)] @ fuse_w  [384, 64]
    import ml_dtypes
    Wi = [fc_w[:, 128 * i:128 * (i + 1)] @ fuse_w for i in range(3)]
    # wout12: branch-pair lhsT for X/Y (K=128): rows 0-63 = branch1 (o2)
    # channels, rows 64-127 = branch2 (o3); cols = 3 mt tiles of 128.
    wout12 = np.zeros((128, 3 * 128), dtype=np.float32)
    wout12[0:64, :] = Wi[1].T.reshape(64, COUT)
    wout12[64:128, :] = Wi[2].T.reshape(64, COUT)
    wout12 = wout12.astype(ml_dtypes.bfloat16)
    # wout1: o1 lhsT (K=64 slices per sample half)
    wout1 = np.zeros((128, COUT), dtype=np.float32)
    wout1[0:64, :] = Wi[0].T
    wout1[64:128, :] = Wi[0].T
    wout1 = wout1.astype(ml_dtypes.bfloat16)
    bias_out = (fc_w @ np.tile(fuse_b, 3) + fc_b).astype(np.float32)  # [384]
    biasout = bias_out.reshape(3, 128).T.copy()   # [128, 3], col mt

    # conv lhsT: 4 block-diag [128, 128] groups; group kc contracts input
    # chans [64kc, 64kc+64) for both samples at once (partition halves).
    wconv = np.zeros((128, 512), dtype=np.float32)
    for kc in range(4):
        wt = conv_w[:, 64 * kc:64 * (kc + 1)].T    # [64 in, 64 out]
        wconv[0:64, 128 * kc:128 * kc + 64] = wt
        wconv[64:128, 128 * kc + 64:128 * (kc + 1)] = wt
    wconv = wconv.astype(ml_dtypes.bfloat16)

    convb = np.concatenate([conv_b, conv_b]).reshape(128, 1).astype(np.float32)
    ident = np.eye(128, dtype=np.float32)

    nc = _get_program()
    in_maps = []
    xbf = x.reshape(NB, CIN, PIX).astype(ml_dtypes.bfloat16)
    for core in range(N_CORES):
        xs = xbf[core * SAMPLES_PER_CORE:(core + 1) * SAMPLES_PER_CORE]
        in_maps.append({
            "x4": np.ascontiguousarray(xs),
            "wconv": wconv, "wout12": wout12, "wout1": wout1,
            "aT": aT, "bT": bT,
            "ident": ident, "convb": convb, "biasout": biasout,
        })
    res = run_bass_kernel_spmd(nc, in_maps, list(range(N_CORES)))
    out = np.empty((NB, COUT, H, W), dtype=np.float32)
    for core in range(N_CORES):
        out[core * SAMPLES_PER_CORE:(core + 1) * SAMPLES_PER_CORE] = (
            res.results[core]["y4"].reshape(SAMPLES_PER_CORE, COUT, H, W))
    return out



# revision 8
# speedup vs baseline: 1.0255x; 1.0255x over previous
"""Trainium2 Bass kernel for nn_DIDAModuleD4 (dynamic depthwise conv module).

Data-parallel over batch: 32 samples -> 8 cores x 4 samples.
Per core, samples are processed in 2 blocks of 2 samples; each block maps the
2x64=128 (sample, channel) pairs onto the 128 SBUF partitions.

Math (per sample, with host-side weight folding):
  f   = relu(conv_w @ x + conv_b)                       [64, 4096]
  g   = relu(mean_px(conv_w @ x + conv_b))              [64]
  k_t = a_t * g + b_t            (43 taps, a/b host-folded scalars)
  o_i = sum_t k_t * shift_t(f)   (depthwise; 5x5, 3x3 d2, 3x3 d4)
  out = sum_i W_i @ o_i + bias_out                      [384, 4096]

All 43 depthwise taps run on the PE as fp8e4 DoubleRow matmuls, two taps per
matmul: f is stored as an fp8 padded tile (72x72 flat domain) plus 6 shifted
copies (SBUF->SBUF DMA, shifts 1/2/4/72/144/288); a tap pair (t, t+delta)
reads k-tiles (slot0@off, slot_delta@off) so the rhs AP is [p, 2, N] with a
large monotonic dim-1 stride (small strides fault the PE).  lhsT k-tiles are
fp8 diag(k_t) matrices built per block from ktile; odd taps pair with an
all-zero diag slot.  DoubleRow costs 0.5 PE cycles/output-column for 2 taps
vs 1.0 for one bf16 tap (4x).  Tap matmuls produce 7-row x 72-col psum
chunks; the 8 pad columns per row are skipped at evacuation (pad wraparound
reads land in neighbor-row pad zeros since |dx*dil| <= PAD).

k values (~2.5e-3) sit in fp8's subnormal range, so ktile is prescaled by a
host-computed power of two per branch and the branch evacuation divides it
back out.  conv (f32r block-diag, 2-sample) and output 1x1s (bf16) are as in
the bf16 baseline; evacuations are spread across ACT/DVE/Pool.
"""

import sys

if "/opt/trn_rl_repo" not in sys.path:
    sys.path.insert(0, "/opt/trn_rl_repo")

import numpy as np
from contextlib import ExitStack

from concourse import bass, mybir, tile, bacc
from concourse.bass_utils import run_bass_kernel_spmd

F32 = mybir.dt.float32
F32R = mybir.dt.float32r
BF16 = mybir.dt.bfloat16
FP8 = mybir.dt.float8e4
AF = mybir.ActivationFunctionType
ALU = mybir.AluOpType
DRMODE = mybir.MatmulPerfMode.DoubleRow

N_CORES = 8
SAMPLES_PER_CORE = 4
CM = 64
CIN = 256
COUT = 384
H = W = 64
PIX = H * W          # 4096
PAD = 4
WP = W + 2 * PAD     # 72
FSZ = WP * WP        # 5184
GUARD = 4            # front/back guard elems per FF slot (OOB garbage ok)
SLP = FSZ + 2 * GUARD  # 5192 slot pitch
SHIFTS = (0, 1, 2, 4, WP, 2 * WP, 4 * WP)   # FF slot shifts
SHIFT_SLOT = {s: i for i, s in enumerate(SHIFTS)}
NFF = len(SHIFTS)    # 7
HALF = 2048          # pixels per half (32 rows)
CHUNK = 512          # conv/out matmul N
SLAB = 1024
NSLAB = PIX // SLAB
# tap-psum chunks per half: 7-row pieces of the 32 rows
TAPCH = ((0, 7), (7, 7), (14, 7), (21, 7), (28, 4))

# taps: (branch, dy, dx); dil = (1, 2, 4)[branch]
DILS = (1, 2, 4)


def _tap_pairs():
    """Pair taps so each pair's flat-offset delta is one of SHIFTS[1:].
    Returns list of (t1, t2_or_None, shift) with t=(br, dy, dx);
    t2's flat offset == t1's + shift (shift==0 for singles)."""
    pairs = []
    # b0 (5x5, dil 1)
    for dy in range(-2, 3):
        pairs.append(((0, dy, -2), (0, dy, -1), 1))
        pairs.append(((0, dy, 0), (0, dy, 1), 1))
    pairs.append(((0, -2, 2), (0, -1, 2), WP))
    pairs.append(((0, 0, 2), (0, 1, 2), WP))
    pairs.append(((0, 2, 2), None, 0))
    # b1 (3x3, dil 2)
    for dy in range(-1, 2):
        pairs.append(((1, dy, -1), (1, dy, 0), 2))
    pairs.append(((1, -1, 1), (1, 0, 1), 2 * WP))
    pairs.append(((1, 1, 1), None, 0))
    # b2 (3x3, dil 4)
    for dy in range(-1, 2):
        pairs.append(((2, dy, -1), (2, dy, 0), 4))
    pairs.append(((2, -1, 1), (2, 0, 1), 4 * WP))
    pairs.append(((2, 1, 1), None, 0))
    return pairs


PAIRS = _tap_pairs()
NSLOT = 2 * len(PAIRS)          # diag bank slots (46)
BR_PAIRS = {br: [(j, p) for j, p in enumerate(PAIRS) if p[0][0] == br]
            for br in range(3)}
assert [len(BR_PAIRS[b]) for b in range(3)] == [13, 5, 5]


def _tap_off(t, row0):
    """Flat offset (within a slot, before the +GUARD base) of tap t's rhs
    for an output chunk starting at block row `row0`, extended col 0."""
    br, dy, dx = t
    dil = DILS[br]
    return (PAD + row0 + dy * dil) * WP + dx * dil


_PROGRAM_CACHE = {}


def _build_program():
    nc = bacc.Bacc("TRN2", target_bir_lowering=False, debug=False,
                   num_devices=N_CORES)

    x4 = nc.dram_tensor("x4", [SAMPLES_PER_CORE, CIN, PIX], BF16,
                        kind="ExternalInput").ap()
    wconv = nc.dram_tensor("wconv", [128, 512], BF16,
                           kind="ExternalInput").ap()
    wout12_d = nc.dram_tensor("wout12", [128, 3 * 128], BF16,
                              kind="ExternalInput").ap()
    wout1_d = nc.dram_tensor("wout1", [128, COUT], BF16,
                             kind="ExternalInput").ap()
    aT_d = nc.dram_tensor("aT", [128, NSLOT], F32, kind="ExternalInput").ap()
    bT_d = nc.dram_tensor("bT", [128, NSLOT], F32, kind="ExternalInput").ap()
    ident_d = nc.dram_tensor("ident", [128, 128], FP8,
                             kind="ExternalInput").ap()
    convb_d = nc.dram_tensor("convb", [128, 1], F32, kind="ExternalInput").ap()
    biasout_d = nc.dram_tensor("biasout", [128, 3], F32,
                               kind="ExternalInput").ap()
    # per-branch inverse tap scales (folded into branch evac)
    sinv_d = nc.dram_tensor("sinv", [128, 3], F32, kind="ExternalInput").ap()
    y4 = nc.dram_tensor("y4", [SAMPLES_PER_CORE, COUT, PIX], F32,
                        kind="ExternalOutput").ap()

    with tile.TileContext(nc) as tc:
        with ExitStack() as ctx:
            consts = ctx.enter_context(tc.tile_pool(name="consts", bufs=1))
            xpool = ctx.enter_context(tc.tile_pool(name="xp", bufs=3))
            ffpool = ctx.enter_context(tc.tile_pool(name="ffp", bufs=1))
            dgpool = ctx.enter_context(tc.tile_pool(name="dgp", bufs=1))
            opool = ctx.enter_context(tc.tile_pool(name="op", bufs=2))
            outpool = ctx.enter_context(tc.tile_pool(name="outp", bufs=2))
            smalls = ctx.enter_context(tc.tile_pool(name="sm", bufs=2))
            ps_a = ctx.enter_context(
                tc.tile_pool(name="psa", bufs=1, space="PSUM"))
            ps_out = ctx.enter_context(
                tc.tile_pool(name="pso", bufs=1, space="PSUM"))

            # ---- constants ----
            wconv_t = consts.tile([128, 512], BF16, tag="wconv")
            nc.gpsimd.dma_start(wconv_t[:], wconv[:])
            wout12_t = consts.tile([128, 3 * 128], BF16, tag="wout12")
            nc.gpsimd.dma_start(wout12_t[:], wout12_d[:])
            wout1_t = consts.tile([128, COUT], BF16, tag="wout1")
            nc.gpsimd.dma_start(wout1_t[:], wout1_d[:])
            aT = consts.tile([128, NSLOT], F32, tag="aT")
            nc.gpsimd.dma_start(aT[:], aT_d[:])
            bT = consts.tile([128, NSLOT], F32, tag="bT")
            nc.gpsimd.dma_start(bT[:], bT_d[:])
            ident = consts.tile([128, 128], FP8, tag="ident")
            nc.gpsimd.dma_start(ident[:], ident_d[:])
            convb = consts.tile([128, 1], F32, tag="convb")
            nc.gpsimd.dma_start(convb[:], convb_d[:])
            biasout = consts.tile([128, 3], F32, tag="biasout")
            nc.gpsimd.dma_start(biasout[:], biasout_d[:])
            sinv = consts.tile([128, 3], F32, tag="sinv")
            nc.gpsimd.dma_start(sinv[:], sinv_d[:])

            # persistent FF tiles (one per block parity): 7 slots of padded
            # fp8 f (slot 0) and its shifted copies; pad borders zeroed once.
            ffs, banks = [], []
            zeros = consts.tile([128, PAD * WP], F32, tag="zeros")
            nc.gpsimd.memset(zeros[:], 0.0)
            for par in range(2):
                ff = ffpool.tile([128, NFF * SLP], FP8, tag=f"ff{par}")
                s0 = ff[:, GUARD:GUARD + FSZ]
                v = s0.rearrange("p (r c) -> p r c", c=WP)
                nc.vector.tensor_copy(s0[:, 0:PAD * WP], zeros[:])
                nc.vector.tensor_copy(s0[:, (PAD + H) * WP:FSZ], zeros[:])
                zv = zeros[:].rearrange("p (r c) -> p r c", c=PAD)
                nc.vector.tensor_copy(v[:, PAD:PAD + H, 0:PAD], zv[:, 0:H, :])
                nc.vector.tensor_copy(v[:, PAD:PAD + H, PAD + W:WP],
                                      zv[:, 0:H, :])
                # zero guards + shifted-slot tails once: stale SBUF there can
                # be Inf/NaN bit patterns, and 0 * Inf = NaN even through a
                # zero diag k-tile
                nc.gpsimd.memset(ff[:, 0:GUARD], 0.0)
                nc.gpsimd.memset(ff[:, GUARD + FSZ:SLP], 0.0)
                for si in range(1, NFF):
                    nc.gpsimd.memset(
                        ff[:, si * SLP + SLP - SHIFTS[si]:(si + 1) * SLP],
                        0.0)
                ffs.append(ff)
                bank = dgpool.tile([128, NSLOT, 128], FP8, tag=f"bank{par}")
                nc.gpsimd.memset(bank[:], 0.0)
                banks.append(bank)

            def ff_slot0_rows(ff, r0, nrows):
                """[p, nrows, 64] valid-interior view of slot0 (conv dst)."""
                v = ff[:, GUARD:GUARD + FSZ].rearrange("p (r c) -> p r c",
                                                       c=WP)
                return v[:, PAD + r0:PAD + r0 + nrows, PAD:PAD + W]

            def pair_rhs(ff, t1, shift, row0, ncols):
                """[p, 2, ncols] rhs AP: k-tile1 = slot0 @ off(t1),
                k-tile2 = slot(shift) @ same off (== f @ off+shift)."""
                off = GUARD + _tap_off(t1, row0)
                v = ff[:, off:off + ncols]
                u = v.unsqueeze(1).broadcast_to([128, 2, ncols])
                # singles (shift 0) pair with the zero diag; point k-tile2
                # at slot 1 — small or zero dim-1 strides fault the PE
                slot = SHIFT_SLOT[shift] or 1
                u.ap[1] = [slot * SLP, 2]
                return u

            # round-robin engine pickers for evac/diag work
            def rr(seq):
                i = [0]

                def pick():
                    e = seq[i[0] % len(seq)]
                    i[0] += 1
                    return e
                return pick

            # Pool/GPSIMD cannot read PSUM: psum evacs go to ACT/DVE only;
            # Pool absorbs SBUF-side work (diags, relu) instead.
            diag_eng = rr(["g", "v", "g", "a", "g"])
            evac_eng = rr(["a", "v"])
            out_eng = rr(["a", "a", "v"])

            def emit_conv(blk):
                n0 = 2 * blk
                ff = ffs[blk % 2]
                bank = banks[blk % 2]

                gsums = smalls.tile([128, 8], F32, tag=f"gsums{blk}")
                for q in range(NSLAB):
                    xts = {}
                    for kc in range(4):
                        xt = xpool.tile([128, SLAB], BF16, tag=f"x{kc}")
                        nc.sync.dma_start(
                            xt[:],
                            x4[n0:n0 + 2, kc * 64:(kc + 1) * 64,
                               q * SLAB:(q + 1) * SLAB])
                        xts[kc] = xt
                    for c in range(SLAB // CHUNK):
                        j = q * (SLAB // CHUNK) + c
                        ps = ps_a.tile([128, CHUNK], F32, tag=f"tap{j % 2}")
                        for kc in range(4):
                            nc.tensor.matmul(
                                ps[:],
                                wconv_t[:, kc * 128:(kc + 1) * 128],
                                xts[kc][:, c * CHUNK:(c + 1) * CHUNK],
                                start=(kc == 0), stop=(kc == 3))
                        dst = ff_slot0_rows(ff, 8 * j, 8)
                        nc.scalar.activation(
                            dst, ps[:], AF.Identity,
                            bias=convb[:, 0:1],
                            accum_out=gsums[:, j:j + 1])

                # relu in place over the full slot0 (pads stay 0), split
                # between DVE and Pool
                s0 = ff[:, GUARD:GUARD + FSZ]
                hf = FSZ // 2
                nc.vector.tensor_scalar_max(s0[:, 0:hf], s0[:, 0:hf], 0.0)
                nc.gpsimd.tensor_scalar_max(s0[:, hf:FSZ], s0[:, hf:FSZ], 0.0)

                # shifted copies via SBUF->SBUF DMA
                for si in range(1, NFF):
                    sh = SHIFTS[si]
                    nc.sync.dma_start(
                        ff[:, si * SLP:si * SLP + SLP - sh],
                        ff[:, sh:SLP])

                # g -> ktile -> diag bank
                gpre = smalls.tile([128, 1], F32, tag=f"gpre{blk}")
                nc.vector.tensor_reduce(gpre[:], gsums[:], op=ALU.add,
                                        axis=mybir.AxisListType.X)
                gt = smalls.tile([128, 1], F32, tag=f"g{blk}")
                nc.scalar.activation(gt[:], gpre[:], AF.Relu,
                                     scale=1.0 / PIX)
                ktile = smalls.tile([128, NSLOT], F32, tag=f"ktile{blk}")
                nc.vector.scalar_tensor_tensor(ktile[:], aT[:], gt[:, 0:1],
                                               bT[:], op0=ALU.mult,
                                               op1=ALU.add)
                nc.vector.tensor_scalar_min(ktile[:], ktile[:], 240.0)
                nc.vector.tensor_scalar_max(ktile[:], ktile[:], -240.0)

                for j, (t1, t2, _sh) in enumerate(PAIRS):
                    for half_, t in ((0, t1), (1, t2)):
                        if t is None:
                            continue
                        sl = 2 * j + half_
                        e = diag_eng()
                        if e == "v":
                            nc.vector.tensor_scalar_mul(
                                bank[:, sl, :], ident[:],
                                ktile[:, sl:sl + 1])
                        elif e == "g":
                            nc.gpsimd.tensor_scalar_mul(
                                bank[:, sl, :], ident[:],
                                ktile[:, sl:sl + 1])
                        else:
                            nc.scalar.activation(
                                bank[:, sl, :], ident[:], AF.Copy,
                                scale=ktile[:, sl:sl + 1])
                return (ff, bank, n0)

            def emit_taps(st, h):
                ff, bank, n0 = st
                o1t = opool.tile([128, HALF], BF16, tag="o1")
                Xt = opool.tile([128, HALF], BF16, tag="X")
                Yt = opool.tile([128, HALF], BF16, tag="Y")
                pi = 0
                for (lr0, nr) in TAPCH:
                    row0 = 32 * h + lr0
                    ncols = nr * WP
                    for br in range(3):
                        ps = ps_a.tile([128, 512], F32, tag=f"tap{pi % 2}")
                        pi += 1
                        plist = BR_PAIRS[br]
                        for i, (j, (t1, t2, sh)) in enumerate(plist):
                            rhs = pair_rhs(ff, t1, sh, row0, ncols)
                            nc.tensor.matmul(
                                ps[:, 0:ncols], bank[:, 2 * j:2 * j + 2, :],
                                rhs, start=(i == 0),
                                stop=(i == len(plist) - 1),
                                perf_mode=DRMODE)
                        # evac: skip the 8 pad cols per row; scale 1/S_br
                        src = ps[:, 0:ncols].rearrange("p (r c) -> p r c",
                                                       c=WP)[:, :, PAD:PAD + W]
                        c0 = lr0 * W
                        csl = slice(c0, c0 + nr * W)
                        if br == 0:
                            dsts = [(slice(0, 128), o1t[:, csl])]
                        elif br == 1:
                            dsts = [(slice(0, 64), Xt[0:64, csl]),
                                    (slice(64, 128), Yt[0:64, csl])]
                        else:
                            dsts = [(slice(0, 64), Xt[64:128, csl]),
                                    (slice(64, 128), Yt[64:128, csl])]
                        for psl, dst in dsts:
                            e = evac_eng()
                            sc = sinv[psl, br:br + 1]
                            if e == "a":
                                nc.scalar.activation(dst, src[psl], AF.Copy,
                                                     scale=sc)
                            elif e == "v":
                                nc.vector.tensor_scalar_mul(dst, src[psl], sc)
                            else:
                                nc.gpsimd.tensor_scalar_mul(dst, src[psl], sc)
                return (n0, h, o1t, Xt, Yt)

            def emit_out(st):
                n0, h, o1t, Xt, Yt = st
                for g2 in range(2):           # 1024-px store groups
                    osbs = {}
                    for mt in range(3):
                        for s in range(2):
                            osb_tile = outpool.tile(
                                [128, 2 * CHUNK], F32, tag=f"osb{mt}_{s}")
                            osbs[(mt, s)] = osb_tile
                    for cc in range(2):
                        c = g2 * 2 + cc
                        csl = slice(c * CHUNK, (c + 1) * CHUNK)
                        pss = {}
                        for mt in range(3):
                            for s, bt in ((0, Xt), (1, Yt)):
                                ps = ps_out.tile([128, CHUNK], F32,
                                                 tag=f"out{s}_{mt}")
                                pss[(s, mt)] = ps
                                nc.tensor.matmul(
                                    ps[:],
                                    wout12_t[:, mt * 128:(mt + 1) * 128],
                                    bt[:, csl], start=True, stop=False)
                        for mt in range(3):
                            for s in range(2):
                                sl = slice(64 * s, 64 * s + 64)
                                nc.tensor.matmul(
                                    pss[(s, mt)][:],
                                    wout1_t[sl, mt * 128:(mt + 1) * 128],
                                    o1t[sl, csl], start=False, stop=True)
                        for mt in range(3):
                            for s in range(2):
                                dst = osbs[(mt, s)][:, cc * CHUNK:
                                                    (cc + 1) * CHUNK]
                                if out_eng() == "a":
                                    nc.scalar.activation(
                                        dst, pss[(s, mt)][:], AF.Identity,
                                        bias=biasout[:, mt:mt + 1])
                                else:
                                    nc.vector.scalar_tensor_tensor(
                                        dst, pss[(s, mt)][:], 1.0,
                                        biasout[:, mt:mt + 1]
                                        .broadcast_to([128, CHUNK]),
                                        op0=ALU.mult, op1=ALU.add)
                    px0 = h * HALF + g2 * 2 * CHUNK
                    for mt in range(3):
                        for s in range(2):
                            n = n0 + s
                            dst = y4[n, mt * 128:(mt + 1) * 128,
                                     px0:px0 + 2 * CHUNK]
                            if g2 == 0:
                                nc.gpsimd.dma_start(dst, osbs[(mt, s)][:])
                            else:
                                nc.scalar.dma_start(dst, osbs[(mt, s)][:])

            st0 = emit_conv(0)
            p00 = emit_taps(st0, 0)
            st1 = emit_conv(1)
            p01 = emit_taps(st0, 1)
            emit_out(p00)
            p10 = emit_taps(st1, 0)
            emit_out(p01)
            p11 = emit_taps(st1, 1)
            emit_out(p10)
            emit_out(p11)
    nc.compile()
    return nc


def _get_program():
    if "nc" not in _PROGRAM_CACHE:
        _PROGRAM_CACHE["nc"] = _build_program()
    return _PROGRAM_CACHE["nc"]


def kernel(x, conv_w, conv_b, ck_w, ck_b, ck2_w, ck2_b, ckd4_w, ckd4_b,
           kern_w, kern_b, kern2_w, kern2_b, kernd4_w, kernd4_b,
           fuse_w, fuse_b, fc_w, fc_b):
    import ml_dtypes
    x = np.asarray(x, dtype=np.float32)
    conv_w = np.asarray(conv_w, dtype=np.float32)
    conv_b = np.asarray(conv_b, dtype=np.float32)
    fuse_w = np.asarray(fuse_w, dtype=np.float32)
    fuse_b = np.asarray(fuse_b, dtype=np.float32)
    fc_w = np.asarray(fc_w, dtype=np.float32)
    fc_b = np.asarray(fc_b, dtype=np.float32)

    NB = x.shape[0]
    assert NB == N_CORES * SAMPLES_PER_CORE

    # tap affine coefficients per branch: k_t = a_t * g + b_t
    def fold(sw, sb, kw, kb):
        a = (float(sw) * np.asarray(kw)).astype(np.float32)
        b = (float(sw) * np.asarray(kb) + float(sb)).astype(np.float32)
        return a, b

    a1, b1 = fold(ck_w, ck_b, kern_w, kern_b)        # [25], 5x5 row-major
    a2, b2 = fold(ck2_w, ck2_b, kern2_w, kern2_b)    # [9]
    a3, b3 = fold(ckd4_w, ckd4_b, kernd4_w, kernd4_b)

    def coef(t):
        br, dy, dx = t
        if br == 0:
            return a1[(dy + 2) * 5 + (dx + 2)], b1[(dy + 2) * 5 + (dx + 2)]
        a, b = (a2, b2) if br == 1 else (a3, b3)
        return a[(dy + 1) * 3 + (dx + 1)], b[(dy + 1) * 3 + (dx + 1)]

    # per-branch power-of-2 prescale: bound |k| with g <= GMAX, keep
    # S*|k| <= 200 so fp8e4m3 never saturates
    GMAX = 1.0
    scales = []
    for br in range(3):
        taps = [coef(t1) for (t1, t2, _s) in PAIRS if t1[0] == br]
        taps += [coef(t2) for (t1, t2, _s) in PAIRS
                 if t2 is not None and t2[0] == br]
        bound = max(abs(a) * GMAX + abs(b) for a, b in taps)
        scales.append(2.0 ** np.floor(np.log2(200.0 / max(bound, 1e-30))))
    sinv = np.zeros((128, 3), np.float32)
    for br in range(3):
        sinv[:, br] = 1.0 / scales[br]

    aT = np.zeros((128, NSLOT), np.float32)
    bT = np.zeros((128, NSLOT), np.float32)
    for j, (t1, t2, _sh) in enumerate(PAIRS):
        for half_, t in ((0, t1), (1, t2)):
            if t is None:
                continue
            a, b = coef(t)
            s = scales[t[0]]
            aT[:, 2 * j + half_] = a * s
            bT[:, 2 * j + half_] = b * s

    # folded output weights W_i = fc_w[:, 128i:128(i+1)] @ fuse_w  [384, 64]
    Wi = [fc_w[:, 128 * i:128 * (i + 1)] @ fuse_w for i in range(3)]
    wout12 = np.zeros((128, 3 * 128), dtype=np.float32)
    wout12[0:64, :] = Wi[1].T.reshape(64, COUT)
    wout12[64:128, :] = Wi[2].T.reshape(64, COUT)
    wout12 = wout12.astype(ml_dtypes.bfloat16)
    wout1 = np.zeros((128, COUT), dtype=np.float32)
    wout1[0:64, :] = Wi[0].T
    wout1[64:128, :] = Wi[0].T
    wout1 = wout1.astype(ml_dtypes.bfloat16)
    bias_out = (fc_w @ np.tile(fuse_b, 3) + fc_b).astype(np.float32)
    biasout = bias_out.reshape(3, 128).T.copy()

    wconv = np.zeros((128, 512), dtype=np.float32)
    for kc in range(4):
        wt = conv_w[:, 64 * kc:64 * (kc + 1)].T
        wconv[0:64, 128 * kc:128 * kc + 64] = wt
        wconv[64:128, 128 * kc + 64:128 * (kc + 1)] = wt
    wconv = wconv.astype(ml_dtypes.bfloat16)

    convb = np.concatenate([conv_b, conv_b]).reshape(128, 1).astype(np.float32)
    ident = np.eye(128, dtype=np.float32).astype(ml_dtypes.float8_e4m3)

    nc = _get_program()
    in_maps = []
    xbf = x.reshape(NB, CIN, PIX).astype(ml_dtypes.bfloat16)
    for core in range(N_CORES):
        xs = xbf[core * SAMPLES_PER_CORE:(core + 1) * SAMPLES_PER_CORE]
        in_maps.append({
            "x4": np.ascontiguousarray(xs),
            "wconv": wconv, "wout12": wout12, "wout1": wout1,
            "aT": aT, "bT": bT, "ident": ident, "convb": convb,
            "biasout": biasout, "sinv": sinv,
        })
    res = run_bass_kernel_spmd(nc, in_maps, list(range(N_CORES)))
    out = np.empty((NB, COUT, H, W), dtype=np.float32)
    for core in range(N_CORES):
        out[core * SAMPLES_PER_CORE:(core + 1) * SAMPLES_PER_CORE] = (
            res.results[core]["y4"].reshape(SAMPLES_PER_CORE, COUT, H, W))
    return out
